# revision 2
# baseline (speedup 1.0000x reference)
"""Trainium2 Bass kernel for nn_AdditiveAttention (B=4, Q=K=2048, D=1024).

Math: scores[b,q,k] = (sum_d q[b,q,d] + sum_d v[b,k,d]) / sqrt(D) + mask bias.
The q-row term is constant along k, so it cancels in the softmax exactly:
    weights[b,q,k] = mask[b,q,k] * e[b,k] / denom[b,q]
    e[b,k]     = exp(sum_d value[b,k,d] / sqrt(D))
    denom[b,q] = sum_k mask[b,q,k] * e[b,k]
    context    = (mask @ (e * value)) / denom        (per batch)
(`query` only affects the reference output through float rounding.)

Sharding: 8 cores = (batch b in 0..3) x (query half h in 0..1); each core
computes a (1024, :) slice of both outputs for its batch. No collectives.

On-chip per core:
  - ksum/e from value (DVE reduce + ACT exp), sval = e*value in bf16
  - context: PE matmul, lhsT = mask^T tile (bf16, exact 0/1), rhs = sval
  - weights: DVE tensor_tensor_reduce (me = maskN*ebc, den = row-sum) then
    ACT copy scaled by 1/den; context scaled by 1/den on DVE.
The host supplies the mask in both layouts (bf16, exact) - pure layout prep.
"""

import os
import sys

import numpy as np
import ml_dtypes

for _p in ("/opt/trn_rl_repo", "/root/.axon_site/_ro/trn_rl_repo"):
    if os.path.isdir(_p) and _p not in sys.path:
        sys.path.append(_p)

import concourse.bacc as bacc
import concourse.mybir as mybir
from concourse.tile import TileContext
from concourse.bass_utils import run_bass_kernel_spmd

B, Q, K, D = 4, 2048, 2048, 1024
P = 128
NCORES = 8
QSH = Q // 2          # query rows per core
KT = K // P           # 16 k tiles
QT = QSH // P         # 8 q tiles per core
F32 = mybir.dt.float32
BF16 = mybir.dt.bfloat16

_cache = {}


def _build():
    nc = bacc.Bacc()
    value = nc.declare_dram_parameter("value", [K, D], F32, isOutput=False)
    maskT = nc.declare_dram_parameter("maskT", [K, QSH], BF16, isOutput=False)
    maskN = nc.declare_dram_parameter("maskN", [QSH, K], BF16, isOutput=False)
    ctx_o = nc.declare_dram_parameter("ctx", [QSH, D], F32, isOutput=True)
    wts_o = nc.declare_dram_parameter("wts", [QSH, K], F32, isOutput=True)
    e_dram = nc.dram_tensor("e_scratch", [1, K], BF16)

    AF = mybir.ActivationFunctionType
    ALU = mybir.AluOpType
    AX = mybir.AxisListType

    with TileContext(nc) as tc:
        with (
            tc.tile_pool(name="vpool", bufs=3) as vpool,
            tc.tile_pool(name="sval", bufs=1) as spool,
            tc.tile_pool(name="stats", bufs=1) as stats,
            tc.tile_pool(name="mt", bufs=2) as mtp,
            tc.tile_pool(name="mn", bufs=2) as mnp,
            tc.tile_pool(name="mepool", bufs=2) as mep,
            tc.tile_pool(name="wsb", bufs=2) as wsb,
            tc.tile_pool(name="csb", bufs=2) as csb,
            tc.tile_pool(name="small", bufs=2) as small,
            tc.tile_pool(name="psum", bufs=2, space="PSUM") as pp,
        ):
            ks = stats.tile([P, KT], F32, tag="ks")
            e_f = stats.tile([P, KT], F32, tag="ef")
            e_b = stats.tile([P, KT], BF16, tag="eb")
            ebc = stats.tile([P, K], BF16, tag="ebc")

            # Stage A: e[k] = exp(rowsum(value)/32); sval = e * value (bf16)
            v3 = value.rearrange("(t p) d -> t p d", p=P)
            svs = []
            for t in range(KT):
                vt = vpool.tile([P, D], F32, tag="v")
                nc.sync.dma_start(out=vt[:], in_=v3[t])
                nc.vector.tensor_reduce(
                    ks[:, t : t + 1], vt[:], axis=AX.X, op=ALU.add
                )
                nc.scalar.activation(
                    e_f[:, t : t + 1], ks[:, t : t + 1], AF.Exp, scale=1.0 / 32.0
                )
                sv = spool.tile([P, D], BF16, tag=f"s{t}")
                nc.vector.tensor_scalar_mul(sv[:], vt[:], e_f[:, t : t + 1])
                svs.append(sv)

            # ebc[q_part, k] = e[k] broadcast along partitions, via DRAM bounce
            nc.vector.tensor_copy(e_b[:], e_f[:])
            nc.sync.dma_start(
                out=e_dram[0, :].rearrange("(t p) -> p t", p=P), in_=e_b[:]
            )
            nc.sync.dma_start(
                out=ebc[:], in_=e_dram[0:1, :].partition_broadcast(P)
            )

            # Stage B: per 128-row query tile
            mT4 = maskT.rearrange("(t p) q -> p t q", p=P)
            for j in range(QT):
                qs = slice(j * P, (j + 1) * P)
                mt = mtp.tile([P, KT, P], BF16, tag="mt")
                nc.sync.dma_start(out=mt[:], in_=mT4[:, :, qs])
                mn = mnp.tile([P, K], BF16, tag="mn")
                nc.sync.dma_start(out=mn[:], in_=maskN[qs, :])

                c0 = pp.tile([P, 512], F32, tag="c0")
                c1 = pp.tile([P, 512], F32, tag="c1")
                for t in range(KT):
                    st, sp = (t == 0), (t == KT - 1)
                    nc.tensor.matmul(
                        c0[:], mt[:, t, :], svs[t][:, 0:512], start=st, stop=sp
                    )
                    nc.tensor.matmul(
                        c1[:], mt[:, t, :], svs[t][:, 512:1024], start=st, stop=sp
                    )

                me = mep.tile([P, K], BF16, tag="me")
                den = small.tile([P, 1], F32, tag="den")
                nc.vector.scalar_tensor_tensor(
                    out=me[:],
                    in0=mn[:],
                    scalar=1.0,
                    in1=ebc[:],
                    op0=ALU.mult,
                    op1=ALU.mult,
                    accum_out=den[:],
                )
                r = small.tile([P, 1], F32, tag="r")
                nc.vector.reciprocal(r[:], den[:])

                w = wsb.tile([P, K], F32, tag="w")
                nc.scalar.activation(w[:], me[:], AF.Copy, scale=r[:])
                nc.scalar.dma_start(out=wts_o[qs, :], in_=w[:])

                cs = csb.tile([P, D], F32, tag="cs")
                nc.vector.tensor_scalar_mul(cs[:, 0:512], c0[:], r[:])
                nc.vector.tensor_scalar_mul(cs[:, 512:1024], c1[:], r[:])
                nc.scalar.dma_start(out=ctx_o[qs, :], in_=cs[:])

    nc.compile()
    return nc


def kernel(query, value, attention_mask):
    nc = _cache.get("nc")
    if nc is None:
        nc = _cache["nc"] = _build()

    value = np.ascontiguousarray(np.asarray(value, dtype=np.float32))
    mask = np.asarray(attention_mask)

    in_maps = []
    for c in range(NCORES):
        b, h = divmod(c, 2)
        msub = mask[b, h * QSH : (h + 1) * QSH, :]
        in_maps.append(
            {
                "value": value[b],
                "maskT": np.ascontiguousarray(msub.T).astype(ml_dtypes.bfloat16),
                "maskN": msub.astype(ml_dtypes.bfloat16),
            }
        )

    res = run_bass_kernel_spmd(nc, in_maps, core_ids=list(range(NCORES)))
    _cache["last_results"] = res

    ctx = np.empty((B, Q, D), np.float32)
    wts = np.empty((B, Q, K), np.float32)
    for c in range(NCORES):
        b, h = divmod(c, 2)
        ctx[b, h * QSH : (h + 1) * QSH] = res.results[c]["ctx"]
        wts[b, h * QSH : (h + 1) * QSH] = res.results[c]["wts"]
    return ctx, wts


# revision 3
# speedup vs baseline: 1.0472x; 1.0472x over previous
"""Trainium2 Bass kernel for nn_AdditiveAttention (B=4, Q=K=2048, D=1024).

Math: scores[b,q,k] = (sum_d q[b,q,d] + sum_d v[b,k,d]) / sqrt(D) + mask bias.
The q-row term is constant along k, so it cancels in the softmax exactly:
    weights[b,q,k] = mask[b,q,k] * e[b,k] / denom[b,q]
    e[b,k]     = exp(sum_d value[b,k,d] / sqrt(D))
    denom[b,q] = sum_k mask[b,q,k] * e[b,k]
    context    = (mask @ (e * value)) / denom        (per batch)
(`query` only affects the reference output through float rounding.)

Sharding: 8 cores = (batch b in 0..3) x (query half h in 0..1); each core
computes a (1024, :) slice of both outputs for its batch. No collectives.

Compute dtype: bf16 matmul (mask operand is exactly representable), fp32
accumulate in PSUM; elementwise mostly bf16; outputs shipped bf16 and
upcast to f32 on the host (~0.2% rel err, tolerance is 2e-2).
"""

import os
import sys

import numpy as np
import ml_dtypes

for _p in ("/opt/trn_rl_repo", "/root/.axon_site/_ro/trn_rl_repo"):
    if os.path.isdir(_p) and _p not in sys.path:
        sys.path.append(_p)

import concourse.bacc as bacc
import concourse.mybir as mybir
from concourse.tile import TileContext
from concourse.bass_utils import run_bass_kernel_spmd

B, Q, K, D = 4, 2048, 2048, 1024
P = 128
NCORES = 8
QSH = Q // 2          # query rows per core
KT = K // P           # 16 k tiles
QT = QSH // P         # 8 q tiles per core
F32 = mybir.dt.float32
BF16 = mybir.dt.bfloat16

_cache = {}


def _build():
    nc = bacc.Bacc()
    value = nc.declare_dram_parameter("value", [K, D], BF16, isOutput=False)
    maskT = nc.declare_dram_parameter("maskT", [K, QSH], BF16, isOutput=False)
    maskN = nc.declare_dram_parameter("maskN", [QSH, K], BF16, isOutput=False)
    ctx_o = nc.declare_dram_parameter("ctx", [QSH, D], BF16, isOutput=True)
    wts_o = nc.declare_dram_parameter("wts", [QSH, K], BF16, isOutput=True)
    e_dram = nc.dram_tensor("e_scratch", [1, K], BF16)

    AF = mybir.ActivationFunctionType
    ALU = mybir.AluOpType

    with TileContext(nc) as tc:
        with (
            tc.tile_pool(name="vpool", bufs=1) as vpool,
            tc.tile_pool(name="sval", bufs=1) as spool,
            tc.tile_pool(name="stats", bufs=1) as stats,
            tc.tile_pool(name="scr", bufs=2) as scr,
            tc.tile_pool(name="masks", bufs=1) as masks,
            tc.tile_pool(name="mepool", bufs=2) as mep,
            tc.tile_pool(name="wsb", bufs=3) as wsb,
            tc.tile_pool(name="csb", bufs=3) as csb,
            tc.tile_pool(name="small", bufs=2) as small,
            tc.tile_pool(name="psum", bufs=2, space="PSUM") as pp,
        ):
            ks = stats.tile([P, KT], F32, tag="ks")
            e_f = stats.tile([P, KT], F32, tag="ef")
            e_b = stats.tile([P, KT], BF16, tag="eb")
            ebc = stats.tile([P, K], BF16, tag="ebc")

            # Whole-tensor mask loads: big contiguous runs per descriptor.
            mTa = masks.tile([P, KT, QSH], BF16, tag="mT")
            nc.sync.dma_start(out=mTa[:], in_=maskT.rearrange("(t p) q -> p t q", p=P))
            mNa = masks.tile([P, QT, K], BF16, tag="mN")
            nc.sync.dma_start(out=mNa[:], in_=maskN.rearrange("(j p) k -> p j k", p=P))

            # Stage A: ksum via ACT copy+accum, e = exp(ksum/32), sval = e*value
            v3 = value.rearrange("(t p) d -> t p d", p=P)
            vts, svs = [], []
            for t in range(KT):
                vt = vpool.tile([P, D], BF16, tag=f"v{t}")
                nc.sync.dma_start(out=vt[:], in_=v3[t])
                vts.append(vt)
            for t in range(KT):
                dummy = scr.tile([P, D], BF16, tag="scratch")
                nc.scalar.activation(
                    dummy[:], vts[t][:], AF.Copy, accum_out=ks[:, t : t + 1]
                )
                nc.scalar.activation(
                    e_f[:, t : t + 1], ks[:, t : t + 1], AF.Exp, scale=1.0 / 32.0
                )
                sv = spool.tile([P, D], BF16, tag=f"s{t}")
                nc.vector.tensor_scalar_mul(sv[:], vts[t][:], e_f[:, t : t + 1])
                svs.append(sv)

            # ebc[q_part, k] = e[k] broadcast along partitions, via DRAM bounce
            nc.vector.tensor_copy(e_b[:], e_f[:])
            nc.sync.dma_start(
                out=e_dram[0, :].rearrange("(t p) -> p t", p=P), in_=e_b[:]
            )
            nc.sync.dma_start(
                out=ebc[:], in_=e_dram[0:1, :].partition_broadcast(P)
            )

            # Stage B: per 128-row query tile
            for j in range(QT):
                qs = slice(j * P, (j + 1) * P)

                c0 = pp.tile([P, 512], F32, tag="c0")
                c1 = pp.tile([P, 512], F32, tag="c1")
                for t in range(KT):
                    st, sp = (t == 0), (t == KT - 1)
                    lhsT = mTa[:, t, qs]
                    nc.tensor.matmul(
                        c0[:], lhsT, svs[t][:, 0:512], start=st, stop=sp
                    )
                    nc.tensor.matmul(
                        c1[:], lhsT, svs[t][:, 512:1024], start=st, stop=sp
                    )

                me = mep.tile([P, K], BF16, tag="me")
                den = small.tile([P, 1], F32, tag="den")
                nc.vector.scalar_tensor_tensor(
                    out=me[:],
                    in0=mNa[:, j, :],
                    scalar=1.0,
                    in1=ebc[:],
                    op0=ALU.mult,
                    op1=ALU.mult,
                    accum_out=den[:],
                )
                r = small.tile([P, 1], F32, tag="r")
                nc.vector.reciprocal(r[:], den[:])

                w = wsb.tile([P, K], BF16, tag="w")
                nc.scalar.activation(w[:], me[:], AF.Copy, scale=r[:])
                nc.scalar.dma_start(out=wts_o[qs, :], in_=w[:])

                cs = csb.tile([P, D], BF16, tag="cs")
                nc.vector.tensor_scalar_mul(cs[:, 0:512], c0[:], r[:])
                nc.vector.tensor_scalar_mul(cs[:, 512:1024], c1[:], r[:])
                nc.scalar.dma_start(out=ctx_o[qs, :], in_=cs[:])

    nc.compile()
    return nc


def kernel(query, value, attention_mask):
    nc = _cache.get("nc")
    if nc is None:
        nc = _cache["nc"] = _build()

    value = np.asarray(value, dtype=np.float32)
    mask = np.asarray(attention_mask)

    in_maps = []
    for c in range(NCORES):
        b, h = divmod(c, 2)
        msub = mask[b, h * QSH : (h + 1) * QSH, :]
        in_maps.append(
            {
                "value": value[b].astype(ml_dtypes.bfloat16),
                "maskT": np.ascontiguousarray(msub.T).astype(ml_dtypes.bfloat16),
                "maskN": msub.astype(ml_dtypes.bfloat16),
            }
        )

    res = run_bass_kernel_spmd(nc, in_maps, core_ids=list(range(NCORES)))
    _cache["last_results"] = res

    ctx = np.empty((B, Q, D), np.float32)
    wts = np.empty((B, Q, K), np.float32)
    for c in range(NCORES):
        b, h = divmod(c, 2)
        ctx[b, h * QSH : (h + 1) * QSH] = res.results[c]["ctx"].astype(np.float32)
        wts[b, h * QSH : (h + 1) * QSH] = res.results[c]["wts"].astype(np.float32)
    return ctx, wts


# revision 6
# speedup vs baseline: 1.1979x; 1.1439x over previous
"""Trainium2 Bass kernel for nn_AdditiveAttention (B=4, Q=K=2048, D=1024).

Math: scores[b,q,k] = (sum_d q[b,q,d] + sum_d v[b,k,d]) / sqrt(D) + mask bias.
The q-row term is constant along k, so it cancels in the softmax exactly:
    weights[b,q,k] = mask[b,q,k] * e[b,k] / denom[b,q]
    e[b,k]     = exp(sum_d value[b,k,d] / sqrt(D))
    denom[b,q] = sum_k mask[b,q,k] * e[b,k]
    context    = (mask @ (e * value)) / denom        (per batch)
(`query` only affects the reference output through float rounding.)

Sharding: 8 cores = (batch b in 0..3) x (query half h in 0..1); each core
computes a (1024, :) slice of both outputs for its batch. No collectives.

Compute dtype: bf16 matmul (mask operand is exactly representable), fp32
accumulate in PSUM; elementwise mostly bf16; outputs shipped bf16 and
upcast to f32 on the host (~0.2% rel err, tolerance is 2e-2).
"""

import os
import sys

import numpy as np
import ml_dtypes

for _p in ("/opt/trn_rl_repo", "/root/.axon_site/_ro/trn_rl_repo"):
    if os.path.isdir(_p) and _p not in sys.path:
        sys.path.append(_p)

import concourse.bacc as bacc
import concourse.mybir as mybir
from concourse.tile import TileContext
from concourse.bass_utils import run_bass_kernel_spmd

B, Q, K, D = 4, 2048, 2048, 1024
P = 128
NCORES = 8
QSH = Q // 2          # query rows per core
KT = K // P           # 16 k tiles
QT = QSH // P         # 8 q tiles per core
F32 = mybir.dt.float32
BF16 = mybir.dt.bfloat16

_cache = {}


def _build():
    nc = bacc.Bacc()
    value = nc.declare_dram_parameter("value", [K, D], BF16, isOutput=False)
    maskT = nc.declare_dram_parameter("maskT", [K, QSH], BF16, isOutput=False)
    maskN = nc.declare_dram_parameter("maskN", [QSH, K], BF16, isOutput=False)
    ctx_o = nc.declare_dram_parameter("ctx", [QSH, D], BF16, isOutput=True)
    wts_o = nc.declare_dram_parameter("wts", [QSH, K], BF16, isOutput=True)
    e_dram = nc.dram_tensor("e_scratch", [1, K], BF16)

    AF = mybir.ActivationFunctionType
    ALU = mybir.AluOpType

    with TileContext(nc) as tc:
        with (
            tc.tile_pool(name="vpool", bufs=1) as vpool,
            tc.tile_pool(name="sval", bufs=1) as spool,
            tc.tile_pool(name="stats", bufs=1) as stats,
            tc.tile_pool(name="scr", bufs=2) as scr,
            tc.tile_pool(name="masks", bufs=1) as masks,
            tc.tile_pool(name="mepool", bufs=2) as mep,
            tc.tile_pool(name="wsb", bufs=3) as wsb,
            tc.tile_pool(name="csb", bufs=3) as csb,
            tc.tile_pool(name="small", bufs=2) as small,
            tc.tile_pool(name="psum", bufs=3, space="PSUM") as pp,
        ):
            ks = stats.tile([P, KT], F32, tag="ks")
            e_f = stats.tile([P, KT], F32, tag="ef")
            e_b = stats.tile([P, KT], BF16, tag="eb")
            ebc = stats.tile([P, K], BF16, tag="ebc")

            mTa = masks.tile([P, KT, QSH], BF16, tag="mT")
            mT3 = maskT.rearrange("(t p) q -> p t q", p=P)
            mNa = masks.tile([P, QT, K], BF16, tag="mN")
            mN3 = maskN.rearrange("(j p) k -> p j k", p=P)

            # Stage A, interleaved with per-tile mask loads so the PE can
            # start accumulating as soon as the first tiles land.
            # ksum split across ACT (copy+accum) and DVE (reduce) to halve
            # the serial latency of the e-chain.
            v3 = value.rearrange("(t p) d -> t p d", p=P)
            vts, svs = [], []
            for t in range(KT):
                vt = vpool.tile([P, D], BF16, tag=f"v{t}")
                nc.sync.dma_start(out=vt[:], in_=v3[t])
                vts.append(vt)
                nc.sync.dma_start(out=mTa[:, t, :], in_=mT3[:, t, :])
            for t in range(KT):
                if t % 2 == 0:
                    dummy = scr.tile([P, D], BF16, tag="scratch")
                    nc.scalar.activation(
                        dummy[:], vts[t][:], AF.Copy, accum_out=ks[:, t : t + 1]
                    )
                else:
                    nc.vector.tensor_reduce(
                        ks[:, t : t + 1], vts[t][:], axis=mybir.AxisListType.X,
                        op=ALU.add,
                    )
                nc.scalar.activation(
                    e_f[:, t : t + 1], ks[:, t : t + 1], AF.Exp, scale=1.0 / 32.0
                )
                sv = spool.tile([P, D], BF16, tag=f"s{t}")
                nc.vector.tensor_scalar_mul(sv[:], vts[t][:], e_f[:, t : t + 1])
                svs.append(sv)

            # ebc[q_part, k] = e[k] broadcast along partitions, via DRAM bounce
            nc.vector.tensor_copy(e_b[:], e_f[:])
            nc.sync.dma_start(
                out=e_dram[0, :].rearrange("(t p) -> p t", p=P), in_=e_b[:]
            )
            nc.sync.dma_start(
                out=ebc[:], in_=e_dram[0:1, :].partition_broadcast(P)
            )

            # maskN loads late - only the DVE weights path consumes them.
            for j in range(QT):
                nc.sync.dma_start(out=mNa[:, j, :], in_=mN3[:, j, :])

            # Stage B: per 128-row query tile
            for j in range(QT):
                qs = slice(j * P, (j + 1) * P)

                c0 = pp.tile([P, 512], F32, tag="c0")
                c1 = pp.tile([P, 512], F32, tag="c1")
                for t in range(KT):
                    st, sp = (t == 0), (t == KT - 1)
                    lhsT = mTa[:, t, qs]
                    nc.tensor.matmul(
                        c0[:], lhsT, svs[t][:, 0:512], start=st, stop=sp
                    )
                    nc.tensor.matmul(
                        c1[:], lhsT, svs[t][:, 512:1024], start=st, stop=sp
                    )

                me = mep.tile([P, K], BF16, tag="me")
                den = small.tile([P, 1], F32, tag="den")
                nc.vector.scalar_tensor_tensor(
                    out=me[:],
                    in0=mNa[:, j, :],
                    scalar=1.0,
                    in1=ebc[:],
                    op0=ALU.mult,
                    op1=ALU.mult,
                    accum_out=den[:],
                )
                r = small.tile([P, 1], F32, tag="r")
                nc.vector.reciprocal(r[:], den[:])

                w = wsb.tile([P, K], BF16, tag="w")
                nc.scalar.activation(w[:], me[:], AF.Copy, scale=r[:])
                nc.scalar.dma_start(out=wts_o[qs, :], in_=w[:])

                cs = csb.tile([P, D], BF16, tag="cs")
                nc.vector.tensor_scalar_mul(cs[:, 0:512], c0[:], r[:])
                nc.vector.tensor_scalar_mul(cs[:, 512:1024], c1[:], r[:])
                nc.scalar.dma_start(out=ctx_o[qs, :], in_=cs[:])

    nc.compile()
    return nc


def kernel(query, value, attention_mask):
    nc = _cache.get("nc")
    if nc is None:
        nc = _cache["nc"] = _build()

    value = np.asarray(value, dtype=np.float32)
    mask = np.asarray(attention_mask)

    in_maps = []
    for c in range(NCORES):
        b, h = divmod(c, 2)
        msub = mask[b, h * QSH : (h + 1) * QSH, :]
        in_maps.append(
            {
                "value": value[b].astype(ml_dtypes.bfloat16),
                "maskT": np.ascontiguousarray(msub.T).astype(ml_dtypes.bfloat16),
                "maskN": msub.astype(ml_dtypes.bfloat16),
            }
        )

    res = run_bass_kernel_spmd(nc, in_maps, core_ids=list(range(NCORES)))
    _cache["last_results"] = res

    ctx = np.empty((B, Q, D), np.float32)
    wts = np.empty((B, Q, K), np.float32)
    for c in range(NCORES):
        b, h = divmod(c, 2)
        ctx[b, h * QSH : (h + 1) * QSH] = res.results[c]["ctx"].astype(np.float32)
        wts[b, h * QSH : (h + 1) * QSH] = res.results[c]["wts"].astype(np.float32)
    return ctx, wts


# revision 7
# speedup vs baseline: 1.2726x; 1.0624x over previous
"""Trainium2 Bass kernel for nn_AdditiveAttention (B=4, Q=K=2048, D=1024).

Math: scores[b,q,k] = (sum_d q[b,q,d] + sum_d v[b,k,d]) / sqrt(D) + mask bias.
The q-row term is constant along k, so it cancels in the softmax exactly:
    weights[b,q,k] = mask[b,q,k] * e[b,k] / denom[b,q]
    e[b,k]     = exp(sum_d value[b,k,d] / sqrt(D))
    denom[b,q] = sum_k mask[b,q,k] * e[b,k]
    context    = (mask @ (e * value)) / denom        (per batch)
(`query` only affects the reference output through float rounding.)

Sharding: 8 cores = (batch b in 0..3) x (query half h in 0..1); each core
computes a (1024, :) slice of both outputs for its batch. No collectives.

Compute dtype: bf16 matmul (mask operand is exactly representable), fp32
accumulate in PSUM; elementwise mostly bf16; outputs shipped bf16 and
upcast to f32 on the host (~0.2% rel err, tolerance is 2e-2).
"""

import os
import sys

import numpy as np
import ml_dtypes

for _p in ("/opt/trn_rl_repo", "/root/.axon_site/_ro/trn_rl_repo"):
    if os.path.isdir(_p) and _p not in sys.path:
        sys.path.append(_p)

import concourse.bacc as bacc
import concourse.mybir as mybir
from concourse.tile import TileContext
from concourse.bass_utils import run_bass_kernel_spmd

B, Q, K, D = 4, 2048, 2048, 1024
P = 128
NCORES = 8
QSH = Q // 2          # query rows per core
KT = K // P           # 16 k tiles
QT = QSH // P         # 8 q tiles per core
F32 = mybir.dt.float32
BF16 = mybir.dt.bfloat16

_cache = {}


def _build():
    nc = bacc.Bacc()
    value = nc.declare_dram_parameter("value", [K, D], BF16, isOutput=False)
    maskT = nc.declare_dram_parameter("maskT", [K, QSH], BF16, isOutput=False)
    maskN = nc.declare_dram_parameter("maskN", [QSH, K], BF16, isOutput=False)
    ctx_o = nc.declare_dram_parameter("ctx", [QSH, D], BF16, isOutput=True)
    wts_o = nc.declare_dram_parameter("wts", [QSH, K], BF16, isOutput=True)
    e_dram = nc.dram_tensor("e_scratch", [1, K], BF16)

    AF = mybir.ActivationFunctionType
    ALU = mybir.AluOpType

    with TileContext(nc) as tc:
        with (
            tc.tile_pool(name="vpool", bufs=1) as vpool,
            tc.tile_pool(name="sval", bufs=1) as spool,
            tc.tile_pool(name="stats", bufs=1) as stats,
            tc.tile_pool(name="scr", bufs=2) as scr,
            tc.tile_pool(name="masks", bufs=1) as masks,
            tc.tile_pool(name="mepool", bufs=2) as mep,
            tc.tile_pool(name="wsb", bufs=3) as wsb,
            tc.tile_pool(name="csb", bufs=3) as csb,
            tc.tile_pool(name="small", bufs=2) as small,
            tc.tile_pool(name="psum", bufs=3, space="PSUM") as pp,
            tc.tile_pool(name="psumd", bufs=2, space="PSUM") as ppd,
        ):
            ks = stats.tile([P, KT], F32, tag="ks")
            e_f = stats.tile([P, KT], F32, tag="ef")
            e_b = stats.tile([P, KT], BF16, tag="eb")
            ebc = stats.tile([P, K], BF16, tag="ebc")

            mTa = masks.tile([P, KT, QSH], BF16, tag="mT")
            mT3 = maskT.rearrange("(t p) q -> p t q", p=P)
            mNa = masks.tile([P, QT, K], BF16, tag="mN")
            mN3 = maskN.rearrange("(j p) k -> p j k", p=P)

            # Stage A, interleaved with per-tile mask loads so the PE can
            # start accumulating as soon as the first tiles land.
            # ksum split across ACT (copy+accum) and DVE (reduce) to halve
            # the serial latency of the e-chain.
            v3 = value.rearrange("(t p) d -> t p d", p=P)
            vts, svs = [], []
            for t in range(KT):
                vt = vpool.tile([P, D], BF16, tag=f"v{t}")
                nc.sync.dma_start(out=vt[:], in_=v3[t])
                vts.append(vt)
                nc.sync.dma_start(out=mTa[:, t, :], in_=mT3[:, t, :])
                if t % 2 == 0:
                    nc.sync.dma_start(out=mNa[:, t // 2, :], in_=mN3[:, t // 2, :])
            for t in range(KT):
                if t % 2 == 0:
                    dummy = scr.tile([P, D], BF16, tag="scratch")
                    nc.scalar.activation(
                        dummy[:], vts[t][:], AF.Copy, accum_out=ks[:, t : t + 1]
                    )
                else:
                    nc.vector.tensor_reduce(
                        ks[:, t : t + 1], vts[t][:], axis=mybir.AxisListType.X,
                        op=ALU.add,
                    )
                nc.scalar.activation(
                    e_f[:, t : t + 1], ks[:, t : t + 1], AF.Exp, scale=1.0 / 32.0
                )
                nc.scalar.activation(
                    e_b[:, t : t + 1], ks[:, t : t + 1], AF.Exp, scale=1.0 / 32.0
                )
                sv = spool.tile([P, D], BF16, tag=f"s{t}")
                nc.vector.tensor_scalar_mul(sv[:], vts[t][:], e_f[:, t : t + 1])
                svs.append(sv)

            # ebc[q_part, k] = e[k] broadcast along partitions, via DRAM bounce
            # (gpsimd SWDGE queue: keeps the blocking wait off the sync queue)
            nc.gpsimd.dma_start(
                out=e_dram[0, :].rearrange("(t p) -> p t", p=P), in_=e_b[:]
            )
            nc.gpsimd.dma_start(
                out=ebc[:], in_=e_dram[0:1, :].partition_broadcast(P)
            )

            # Stage B: per 128-row query tile
            for j in range(QT):
                qs = slice(j * P, (j + 1) * P)

                c0 = pp.tile([P, 512], F32, tag="c0")
                c1 = pp.tile([P, 512], F32, tag="c1")
                dn = ppd.tile([P, 1], F32, tag="dn")
                for t in range(KT):
                    st, sp = (t == 0), (t == KT - 1)
                    lhsT = mTa[:, t, qs]
                    nc.tensor.matmul(
                        c0[:], lhsT, svs[t][:, 0:512], start=st, stop=sp
                    )
                    nc.tensor.matmul(
                        c1[:], lhsT, svs[t][:, 512:1024], start=st, stop=sp
                    )
                    nc.tensor.matmul(
                        dn[:], lhsT, e_b[:, t : t + 1], start=st, stop=sp
                    )

                me = mep.tile([P, K], BF16, tag="me")
                nc.vector.tensor_tensor(
                    out=me[:], in0=mNa[:, j, :], in1=ebc[:], op=ALU.mult
                )
                r = small.tile([P, 1], F32, tag="r")
                nc.vector.reciprocal(r[:], dn[:])

                w = wsb.tile([P, K], BF16, tag="w")
                nc.scalar.activation(w[:], me[:], AF.Copy, scale=r[:])
                nc.scalar.dma_start(out=wts_o[qs, :], in_=w[:])

                cs = csb.tile([P, D], BF16, tag="cs")
                nc.vector.tensor_scalar_mul(cs[:, 0:512], c0[:], r[:])
                nc.vector.tensor_scalar_mul(cs[:, 512:1024], c1[:], r[:])
                nc.scalar.dma_start(out=ctx_o[qs, :], in_=cs[:])

    nc.compile()
    return nc


def kernel(query, value, attention_mask):
    nc = _cache.get("nc")
    if nc is None:
        nc = _cache["nc"] = _build()

    value = np.asarray(value, dtype=np.float32)
    mask = np.asarray(attention_mask)

    in_maps = []
    for c in range(NCORES):
        b, h = divmod(c, 2)
        msub = mask[b, h * QSH : (h + 1) * QSH, :]
        in_maps.append(
            {
                "value": value[b].astype(ml_dtypes.bfloat16),
                "maskT": np.ascontiguousarray(msub.T).astype(ml_dtypes.bfloat16),
                "maskN": msub.astype(ml_dtypes.bfloat16),
            }
        )

    res = run_bass_kernel_spmd(nc, in_maps, core_ids=list(range(NCORES)))
    _cache["last_results"] = res

    ctx = np.empty((B, Q, D), np.float32)
    wts = np.empty((B, Q, K), np.float32)
    for c in range(NCORES):
        b, h = divmod(c, 2)
        ctx[b, h * QSH : (h + 1) * QSH] = res.results[c]["ctx"].astype(np.float32)
        wts[b, h * QSH : (h + 1) * QSH] = res.results[c]["wts"].astype(np.float32)
    return ctx, wts


# revision 9
# speedup vs baseline: 1.4480x; 1.1378x over previous
"""Trainium2 Bass kernel for nn_AdditiveAttention (B=4, Q=K=2048, D=1024).

Math: scores[b,q,k] = (sum_d q[b,q,d] + sum_d v[b,k,d]) / sqrt(D) + mask bias.
The q-row term is constant along k, so it cancels in the softmax exactly:
    weights[b,q,k] = mask[b,q,k] * e[b,k] / denom[b,q]
    e[b,k]     = exp(sum_d value[b,k,d] / sqrt(D))
    denom[b,q] = sum_k mask[b,q,k] * e[b,k]
    context    = (mask @ (e * value)) / denom        (per batch)
(`query` only affects the reference output through float rounding.)

Sharding: 8 cores = (batch b in 0..3) x (query half h in 0..1); each core
computes a (1024, :) slice of both outputs for its batch. No collectives.

Compute dtype: bf16 matmul (mask operand is exactly representable), fp32
accumulate in PSUM; elementwise mostly bf16; outputs shipped bf16 and
upcast to f32 on the host (~0.2% rel err, tolerance is 2e-2).
"""

import os
import sys

import numpy as np
import ml_dtypes

for _p in ("/opt/trn_rl_repo", "/root/.axon_site/_ro/trn_rl_repo"):
    if os.path.isdir(_p) and _p not in sys.path:
        sys.path.append(_p)

import concourse.bacc as bacc
import concourse.mybir as mybir
from concourse.tile import TileContext
from concourse.bass_utils import run_bass_kernel_spmd

B, Q, K, D = 4, 2048, 2048, 1024
P = 128
NCORES = 8
QSH = Q // 2          # query rows per core
KT = K // P           # 16 k tiles
QT = QSH // P         # 8 q tiles per core
F32 = mybir.dt.float32
BF16 = mybir.dt.bfloat16

_cache = {}


def _build():
    nc = bacc.Bacc()
    value = nc.declare_dram_parameter("value", [K, D], BF16, isOutput=False)
    maskT = nc.declare_dram_parameter("maskT", [K, QSH], BF16, isOutput=False)
    maskN = nc.declare_dram_parameter("maskN", [QSH, K], BF16, isOutput=False)
    ctx_o = nc.declare_dram_parameter("ctx", [QSH, D], BF16, isOutput=True)
    wts_o = nc.declare_dram_parameter("wts", [QSH, K], BF16, isOutput=True)
    e_dram = nc.dram_tensor("e_scratch", [1, K], BF16)

    AF = mybir.ActivationFunctionType
    ALU = mybir.AluOpType

    with TileContext(nc) as tc:
        with (
            tc.tile_pool(name="vpool", bufs=1) as vpool,
            tc.tile_pool(name="sval", bufs=1) as spool,
            tc.tile_pool(name="stats", bufs=1) as stats,
            tc.tile_pool(name="scr", bufs=2) as scr,
            tc.tile_pool(name="masks", bufs=1) as masks,
            tc.tile_pool(name="mepool", bufs=2) as mep,
            tc.tile_pool(name="wsb", bufs=3) as wsb,
            tc.tile_pool(name="csb", bufs=3) as csb,
            tc.tile_pool(name="small", bufs=8) as small,
            tc.tile_pool(name="psum", bufs=3, space="PSUM") as pp,
            tc.tile_pool(name="psumd", bufs=2, space="PSUM") as ppd,
        ):
            ks = stats.tile([P, KT], F32, tag="ks")
            e_f = stats.tile([P, KT], F32, tag="ef")
            e_b = stats.tile([P, KT], BF16, tag="eb")
            ebc = stats.tile([P, K], BF16, tag="ebc")

            mTa = masks.tile([P, KT, QSH], BF16, tag="mT")
            mT3 = maskT.rearrange("(t p) q -> p t q", p=P)
            mNa = masks.tile([P, QT, K], BF16, tag="mN")
            mN3 = maskN.rearrange("(j p) k -> p j k", p=P)

            # Stage A, interleaved with per-tile mask loads so the PE can
            # start accumulating as soon as the first tiles land.
            # ksum split across ACT (copy+accum) and DVE (reduce) to halve
            # the serial latency of the e-chain.
            v3 = value.rearrange("(t p) d -> t p d", p=P)
            vts, svs = [], []
            for t in range(KT):
                vt = vpool.tile([P, D], BF16, tag=f"v{t}")
                nc.sync.dma_start(out=vt[:], in_=v3[t])
                vts.append(vt)
                nc.sync.dma_start(out=mTa[:, t, :], in_=mT3[:, t, :])
            for t in range(KT):
                if t % 2 == 0:
                    dummy = scr.tile([P, D], BF16, tag="scratch")
                    nc.scalar.activation(
                        dummy[:], vts[t][:], AF.Copy, accum_out=ks[:, t : t + 1]
                    )
                else:
                    nc.vector.tensor_reduce(
                        ks[:, t : t + 1], vts[t][:], axis=mybir.AxisListType.X,
                        op=ALU.add,
                    )
                nc.scalar.activation(
                    e_f[:, t : t + 1], ks[:, t : t + 1], AF.Exp, scale=1.0 / 32.0
                )
                nc.scalar.activation(
                    e_b[:, t : t + 1], ks[:, t : t + 1], AF.Exp, scale=1.0 / 32.0
                )
                sv = spool.tile([P, D], BF16, tag=f"s{t}")
                nc.vector.tensor_scalar_mul(sv[:], vts[t][:], e_f[:, t : t + 1])
                svs.append(sv)

            # maskN after the critical value+maskT stream (weights path only)
            for j in range(QT):
                nc.sync.dma_start(out=mNa[:, j, :], in_=mN3[:, j, :])

            # ebc[q_part, k] = e[k] broadcast along partitions, via DRAM bounce
            # (gpsimd SWDGE queue: keeps the blocking wait off the sync queue)
            nc.gpsimd.dma_start(
                out=e_dram[0, :].rearrange("(t p) -> p t", p=P), in_=e_b[:]
            )
            nc.gpsimd.dma_start(
                out=ebc[:], in_=e_dram[0:1, :].partition_broadcast(P)
            )

            # Stage B pass 1: context matmuls + denominator + evacuation.
            # First two query tiles run their k-chains interleaved so the PE
            # tracks the input DMA stream densely during the ramp.
            cts = {}

            def mm_step(j, t):
                qs = slice(j * P, (j + 1) * P)
                st, sp = (t == 0), (t == KT - 1)
                c0, c1, dn = cts[j]
                lhsT = mTa[:, t, qs]
                nc.tensor.matmul(c0[:], lhsT, svs[t][:, 0:512], start=st, stop=sp)
                nc.tensor.matmul(c1[:], lhsT, svs[t][:, 512:1024], start=st, stop=sp)
                nc.tensor.matmul(dn[:], lhsT, e_b[:, t : t + 1], start=st, stop=sp)

            def evac(j):
                qs = slice(j * P, (j + 1) * P)
                c0, c1, dn = cts.pop(j)
                r = small.tile([P, 1], F32, tag="r", name=f"r_{j}")
                nc.vector.reciprocal(r[:], dn[:])
                cs = csb.tile([P, D], BF16, tag="cs")
                nc.vector.tensor_scalar_mul(cs[:, 0:512], c0[:], r[:])
                nc.vector.tensor_scalar_mul(cs[:, 512:1024], c1[:], r[:])
                nc.scalar.dma_start(out=ctx_o[qs, :], in_=cs[:])
                return r

            rs = {}
            for j in (0, 1):
                cts[j] = (
                    pp.tile([P, 512], F32, tag="c0", name=f"c0_{j}"),
                    pp.tile([P, 512], F32, tag="c1", name=f"c1_{j}"),
                    ppd.tile([P, 1], F32, tag="dn", name=f"dn_{j}"),
                )
            for t in range(KT):
                mm_step(0, t)
                mm_step(1, t)
            rs[0] = evac(0)
            rs[1] = evac(1)
            for j in range(2, QT):
                cts[j] = (
                    pp.tile([P, 512], F32, tag="c0", name=f"c0_{j}"),
                    pp.tile([P, 512], F32, tag="c1", name=f"c1_{j}"),
                    ppd.tile([P, 1], F32, tag="dn", name=f"dn_{j}"),
                )
                for t in range(KT):
                    mm_step(j, t)
                rs[j] = evac(j)

            # Stage B pass 2: attention-weights output (off the critical path)
            for j in range(QT):
                qs = slice(j * P, (j + 1) * P)
                me = mep.tile([P, K], BF16, tag="me")
                nc.vector.tensor_tensor(
                    out=me[:], in0=mNa[:, j, :], in1=ebc[:], op=ALU.mult
                )
                w = wsb.tile([P, K], BF16, tag="w")
                nc.scalar.activation(w[:], me[:], AF.Copy, scale=rs[j][:])
                nc.scalar.dma_start(out=wts_o[qs, :], in_=w[:])

    nc.compile()
    return nc


def kernel(query, value, attention_mask):
    nc = _cache.get("nc")
    if nc is None:
        nc = _cache["nc"] = _build()

    value = np.asarray(value, dtype=np.float32)
    mask = np.asarray(attention_mask)

    in_maps = []
    for c in range(NCORES):
        b, h = divmod(c, 2)
        msub = mask[b, h * QSH : (h + 1) * QSH, :]
        in_maps.append(
            {
                "value": value[b].astype(ml_dtypes.bfloat16),
                "maskT": np.ascontiguousarray(msub.T).astype(ml_dtypes.bfloat16),
                "maskN": msub.astype(ml_dtypes.bfloat16),
            }
        )

    res = run_bass_kernel_spmd(nc, in_maps, core_ids=list(range(NCORES)))
    _cache["last_results"] = res

    ctx = np.empty((B, Q, D), np.float32)
    wts = np.empty((B, Q, K), np.float32)
    for c in range(NCORES):
        b, h = divmod(c, 2)
        ctx[b, h * QSH : (h + 1) * QSH] = res.results[c]["ctx"].astype(np.float32)
        wts[b, h * QSH : (h + 1) * QSH] = res.results[c]["wts"].astype(np.float32)
    return ctx, wts


# revision 10
# speedup vs baseline: 1.5365x; 1.0611x over previous
"""Trainium2 Bass kernel for nn_AdditiveAttention (B=4, Q=K=2048, D=1024).

Math: scores[b,q,k] = (sum_d q[b,q,d] + sum_d v[b,k,d]) / sqrt(D) + mask bias.
The q-row term is constant along k, so it cancels in the softmax exactly:
    weights[b,q,k] = mask[b,q,k] * e[b,k] / denom[b,q]
    e[b,k]     = exp(sum_d value[b,k,d] / sqrt(D))
    denom[b,q] = sum_k mask[b,q,k] * e[b,k]
    context    = (mask @ (e * value)) / denom        (per batch)
(`query` only affects the reference output through float rounding.)

Sharding: 8 cores = (batch b in 0..3) x (query half h in 0..1); each core
computes a (1024, :) slice of both outputs for its batch. No collectives.

Compute dtype: bf16 matmul (mask operand is exactly representable), fp32
accumulate in PSUM; elementwise mostly bf16; outputs shipped bf16 and
upcast to f32 on the host (~0.2% rel err, tolerance is 2e-2).
"""

import os
import sys

import numpy as np
import ml_dtypes

for _p in ("/opt/trn_rl_repo", "/root/.axon_site/_ro/trn_rl_repo"):
    if os.path.isdir(_p) and _p not in sys.path:
        sys.path.append(_p)

import concourse.bacc as bacc
import concourse.mybir as mybir
from concourse.tile import TileContext
from concourse.bass_utils import run_bass_kernel_spmd

B, Q, K, D = 4, 2048, 2048, 1024
P = 128
NCORES = 8
QSH = Q // 2          # query rows per core
KT = K // P           # 16 k tiles
QT = QSH // P         # 8 q tiles per core
F32 = mybir.dt.float32
BF16 = mybir.dt.bfloat16

_cache = {}


def _build():
    nc = bacc.Bacc()
    value = nc.declare_dram_parameter("value", [K, D], BF16, isOutput=False)
    maskT = nc.declare_dram_parameter("maskT", [K, QSH], BF16, isOutput=False)
    maskN = nc.declare_dram_parameter("maskN", [QSH, K], BF16, isOutput=False)
    ctx_o = nc.declare_dram_parameter("ctx", [QSH, D], BF16, isOutput=True)
    wts_o = nc.declare_dram_parameter("wts", [QSH, K], BF16, isOutput=True)
    e_dram = nc.dram_tensor("e_scratch", [1, K], BF16)

    AF = mybir.ActivationFunctionType
    ALU = mybir.AluOpType

    with TileContext(nc) as tc:
        with (
            tc.tile_pool(name="vpool", bufs=1) as vpool,
            tc.tile_pool(name="sval", bufs=1) as spool,
            tc.tile_pool(name="stats", bufs=1) as stats,
            tc.tile_pool(name="scr", bufs=2) as scr,
            tc.tile_pool(name="masks", bufs=1) as masks,
            tc.tile_pool(name="mepool", bufs=3) as mep,
            tc.tile_pool(name="wsb", bufs=3) as wsb,
            tc.tile_pool(name="csb", bufs=3) as csb,
            tc.tile_pool(name="small", bufs=8) as small,
            tc.tile_pool(name="psum", bufs=3, space="PSUM") as pp,
            tc.tile_pool(name="psumd", bufs=2, space="PSUM") as ppd,
        ):
            ks = stats.tile([P, KT], F32, tag="ks")
            e_f = stats.tile([P, KT], F32, tag="ef")
            e_b = stats.tile([P, KT], BF16, tag="eb")
            ebc = stats.tile([P, K], BF16, tag="ebc")

            mTa = masks.tile([P, KT, QSH], BF16, tag="mT")
            mT3 = maskT.rearrange("(t p) q -> p t q", p=P)
            mNa = masks.tile([P, QT, K], BF16, tag="mN")
            mN3 = maskN.rearrange("(j p) k -> p j k", p=P)

            # Stage A, interleaved with per-tile mask loads so the PE can
            # start accumulating as soon as the first tiles land.
            # ksum split across ACT (copy+accum) and DVE (reduce) to halve
            # the serial latency of the e-chain.
            v3 = value.rearrange("(t p) d -> t p d", p=P)
            vts, svs = [], []
            for t in range(KT):
                vt = vpool.tile([P, D], BF16, tag=f"v{t}")
                nc.sync.dma_start(out=vt[:], in_=v3[t])
                vts.append(vt)
                nc.sync.dma_start(out=mTa[:, t, :], in_=mT3[:, t, :])
            for t in range(KT):
                if t % 2 == 0:
                    dummy = scr.tile([P, D], BF16, tag="scratch")
                    nc.scalar.activation(
                        dummy[:], vts[t][:], AF.Copy, accum_out=ks[:, t : t + 1]
                    )
                else:
                    nc.vector.tensor_reduce(
                        ks[:, t : t + 1], vts[t][:], axis=mybir.AxisListType.X,
                        op=ALU.add,
                    )
                nc.scalar.activation(
                    e_f[:, t : t + 1], ks[:, t : t + 1], AF.Exp, scale=1.0 / 32.0
                )
                nc.scalar.activation(
                    e_b[:, t : t + 1], ks[:, t : t + 1], AF.Exp, scale=1.0 / 32.0
                )
                sv = spool.tile([P, D], BF16, tag=f"s{t}")
                nc.vector.tensor_scalar_mul(sv[:], vts[t][:], e_f[:, t : t + 1])
                svs.append(sv)

            # maskN after the critical value+maskT stream (weights path only)
            for j in range(QT):
                nc.sync.dma_start(out=mNa[:, j, :], in_=mN3[:, j, :])

            # ebc[q_part, k] = e[k] broadcast along partitions, via DRAM bounce
            # (sync HWDGE: fast descriptor gen; queue is drained of inputs by
            # the time e_b is ready, so the wait blocks nothing)
            nc.sync.dma_start(
                out=e_dram[0, :].rearrange("(t p) -> p t", p=P), in_=e_b[:]
            )
            nc.sync.dma_start(
                out=ebc[:], in_=e_dram[0:1, :].partition_broadcast(P)
            )

            # Stage B: context matmuls + denominator (PE), evacuation (DVE),
            # weights path (DVE multiply + ACT scale) threaded in between.
            # First three query tiles' c0/c1 chains (and two dn chains) are
            # interleaved so the PE tracks the input DMA stream densely.
            cts, dns, rs = {}, {}, {}

            def alloc_ct(j):
                cts[j] = (
                    pp.tile([P, 512], F32, tag="c0", name=f"c0_{j}"),
                    pp.tile([P, 512], F32, tag="c1", name=f"c1_{j}"),
                )

            def alloc_dn(j):
                dns[j] = ppd.tile([P, 1], F32, tag="dn", name=f"dn_{j}")

            def mm_ct(j, t):
                qs = slice(j * P, (j + 1) * P)
                st, sp = (t == 0), (t == KT - 1)
                c0, c1 = cts[j]
                lhsT = mTa[:, t, qs]
                nc.tensor.matmul(c0[:], lhsT, svs[t][:, 0:512], start=st, stop=sp)
                nc.tensor.matmul(c1[:], lhsT, svs[t][:, 512:1024], start=st, stop=sp)

            def mm_dn(j, t):
                qs = slice(j * P, (j + 1) * P)
                st, sp = (t == 0), (t == KT - 1)
                nc.tensor.matmul(
                    dns[j][:], mTa[:, t, qs], e_b[:, t : t + 1], start=st, stop=sp
                )

            def evac(j):
                qs = slice(j * P, (j + 1) * P)
                c0, c1 = cts.pop(j)
                dn = dns.pop(j)
                r = small.tile([P, 1], F32, tag="r", name=f"r_{j}")
                nc.vector.reciprocal(r[:], dn[:])
                cs = csb.tile([P, D], BF16, tag="cs", name=f"cs_{j}")
                nc.vector.tensor_scalar_mul(cs[:, 0:512], c0[:], r[:])
                nc.vector.tensor_scalar_mul(cs[:, 512:1024], c1[:], r[:])
                nc.scalar.dma_start(out=ctx_o[qs, :], in_=cs[:])
                rs[j] = r

            def wts_path(j):
                qs = slice(j * P, (j + 1) * P)
                me = mep.tile([P, K], BF16, tag="me", name=f"me_{j}")
                nc.vector.tensor_tensor(
                    out=me[:], in0=mNa[:, j, :], in1=ebc[:], op=ALU.mult
                )
                w = wsb.tile([P, K], BF16, tag="w", name=f"w_{j}")
                nc.scalar.activation(w[:], me[:], AF.Copy, scale=rs[j][:])
                nc.scalar.dma_start(out=wts_o[qs, :], in_=w[:])

            RAMP = (0, 1, 2)
            for j in RAMP:
                alloc_ct(j)
            alloc_dn(0)
            alloc_dn(1)
            for t in range(KT):
                for j in RAMP:
                    mm_ct(j, t)
                    if j < 2:
                        mm_dn(j, t)
            alloc_dn(2)
            for t in range(KT):
                mm_dn(2, t)
            for j in RAMP:
                evac(j)
            for j in RAMP:
                wts_path(j)
            for j in range(3, QT):
                alloc_ct(j)
                alloc_dn(j)
                for t in range(KT):
                    mm_ct(j, t)
                    mm_dn(j, t)
                wts_pre = mep.tile([P, K], BF16, tag="me", name=f"me_{j}")
                nc.vector.tensor_tensor(
                    out=wts_pre[:], in0=mNa[:, j, :], in1=ebc[:], op=ALU.mult
                )
                evac(j)
                qs = slice(j * P, (j + 1) * P)
                w = wsb.tile([P, K], BF16, tag="w", name=f"w_{j}")
                nc.scalar.activation(w[:], wts_pre[:], AF.Copy, scale=rs[j][:])
                nc.scalar.dma_start(out=wts_o[qs, :], in_=w[:])

    nc.compile()
    return nc


def kernel(query, value, attention_mask):
    nc = _cache.get("nc")
    if nc is None:
        nc = _cache["nc"] = _build()

    value = np.asarray(value, dtype=np.float32)
    mask = np.asarray(attention_mask)

    in_maps = []
    for c in range(NCORES):
        b, h = divmod(c, 2)
        msub = mask[b, h * QSH : (h + 1) * QSH, :]
        in_maps.append(
            {
                "value": value[b].astype(ml_dtypes.bfloat16),
                "maskT": np.ascontiguousarray(msub.T).astype(ml_dtypes.bfloat16),
                "maskN": msub.astype(ml_dtypes.bfloat16),
            }
        )

    res = run_bass_kernel_spmd(nc, in_maps, core_ids=list(range(NCORES)))
    _cache["last_results"] = res

    ctx = np.empty((B, Q, D), np.float32)
    wts = np.empty((B, Q, K), np.float32)
    for c in range(NCORES):
        b, h = divmod(c, 2)
        ctx[b, h * QSH : (h + 1) * QSH] = res.results[c]["ctx"].astype(np.float32)
        wts[b, h * QSH : (h + 1) * QSH] = res.results[c]["wts"].astype(np.float32)
    return ctx, wts
